# revision 14
# baseline (speedup 1.0000x reference)
"""Trainium2 Bass kernel: nearest triangle (closest point on mesh) brute force.

kernel(triangles [1,1024,3,3] f32, points [1,16384,3] f32) ->
    (distances [1,16384] f32, closest_points [1,16384,3] f32,
     closest_faces [1,16384] int32)

Sharding: data-parallel over points across 8 NeuronCores (2048 points/core);
triangles (and derived per-triangle constants) replicated on every core.
"""

import numpy as np

N_CORES = 8
F = 1024
P_TOTAL = 16384
P_LOC = P_TOTAL // N_CORES          # 2048
PTILES = P_LOC // 128               # 16 point tiles of 128 per core
FH = F // 2                         # 512, PSUM-bank-sized chunk

_PROGRAM_CACHE = {}


def _build_program():
    """Build + compile the (input-independent) Bass program once."""
    import concourse.bacc as bacc
    import concourse.mybir as mybir
    from concourse import tile

    dt = mybir.dt
    alu = mybir.AluOpType
    AF = mybir.ActivationFunctionType

    nc = bacc.Bacc("TRN2", target_bir_lowering=False, debug=False,
                   num_devices=N_CORES)

    d_crows = nc.dram_tensor("crows", [16, F], dt.float32, kind="ExternalInput")
    d_m6 = nc.dram_tensor("m6", [24, F], dt.float32, kind="ExternalInput")
    d_ptsT = nc.dram_tensor("ptsT", [4, P_LOC], dt.float32, kind="ExternalInput")
    d_pcol = nc.dram_tensor("pcol", [128, 3 * PTILES], dt.float32,
                            kind="ExternalInput")
    d_od = nc.dram_tensor("od", [128, PTILES], dt.float32, kind="ExternalOutput")
    d_ocp = nc.dram_tensor("ocp", [128, 3 * PTILES], dt.float32,
                           kind="ExternalOutput")
    d_of = nc.dram_tensor("of", [128, PTILES], dt.float32, kind="ExternalOutput")

    f32 = dt.float32

    with tile.TileContext(nc) as tc:
        with (
            tc.tile_pool(name="const", bufs=1) as cpool,
            tc.tile_pool(name="dwork", bufs=1) as dpool,
            tc.tile_pool(name="work", bufs=1) as wpool,
            tc.tile_pool(name="small", bufs=2) as spool,
            tc.tile_pool(name="psum", bufs=1, space="PSUM") as ppool,
        ):
            V = nc.vector
            G = nc.gpsimd
            S = nc.scalar

            def WT(tag, bufs=None, dtype=None):
                return wpool.tile([128, F], dtype or f32, tag=tag, name=tag,
                                  bufs=bufs)

            # ---- stage inputs ----
            # PE requires operand base partition in {0,32,64}: every matmul
            # operand tile is base-0.
            m6t = []
            for j in range(6):
                mj = cpool.tile([4, F], f32, tag=f"m6_{j}", name=f"m6_{j}")
                G.dma_start(mj[:], d_m6[4 * j:4 * j + 4, :])
                m6t.append(mj)
            pcol = cpool.tile([128, 3 * PTILES], f32, tag="pcol", name="pcol")
            G.dma_start(pcol[:], d_pcol[:])

            ones = cpool.tile([1, 128], f32, tag="ones", name="ones")
            G.memset(ones[:], 1.0)

            # ---- broadcast per-triangle constant rows to [128, F] tiles ----
            # crows rows: ABx ABy ABz ACx ACy ACz Ax Ay Az RAB RAC RBC RDEN IOTA
            CN = ["ABx", "ABy", "ABz", "ACx", "ACy", "ACz",
                  "Ax", "Ay", "Az", "RAB", "RAC", "RBC", "RDEN", "IOTA"]
            CB = {}
            for i, nm in enumerate(CN):
                cs = wpool.tile([1, F], f32, tag="tmpA", name="crowstage")
                G.dma_start(cs[:], d_crows[i:i + 1, :])
                ct = cpool.tile([128, F], f32, tag=nm, name=nm)
                for h in range(2):
                    ps = ppool.tile([128, FH], f32, tag="bps", name="bps", bufs=2)
                    nc.tensor.matmul(ps[:], ones[:],
                                     cs[0:1, h * FH:(h + 1) * FH])
                    S.copy(ct[:, h * FH:(h + 1) * FH], ps[:])
                CB[nm] = ct
            zero = cpool.tile([128, F], f32, tag="ZERO", name="ZERO")
            G.memset(zero[:], 0.0)
            onet = cpool.tile([128, F], f32, tag="ONE", name="ONE")
            G.memset(onet[:], 1.0)

            od = cpool.tile([128, PTILES], f32, tag="od", name="od")
            ocp = cpool.tile([128, 3 * PTILES], f32, tag="ocp", name="ocp")
            of = cpool.tile([128, PTILES], f32, tag="of", name="of")

            ABb = [CB["ABx"], CB["ABy"], CB["ABz"]]
            ACb = [CB["ACx"], CB["ACy"], CB["ACz"]]
            Ab = [CB["Ax"], CB["Ay"], CB["Az"]]

            for t in range(PTILES):
                # ---- d1..d6 via PE homogeneous matmuls ----
                pT = cpool.tile([4, 128], f32, tag="pT", name="pT", bufs=2)
                G.dma_start(pT[:], d_ptsT[0:4, t * 128:(t + 1) * 128])
                ds = []
                for j in range(6):
                    dj = dpool.tile([128, F], f32, tag=f"d{j}", name=f"d{j}")
                    for h in range(2):
                        ps = ppool.tile([128, FH], f32, tag="dps", name="dps", bufs=6)
                        nc.tensor.matmul(
                            ps[:], pT[:],
                            m6t[j][0:4, h * FH:(h + 1) * FH])
                        S.copy(dj[:, h * FH:(h + 1) * FH], ps[:])
                    ds.append(dj)
                d1, d2, d3, d4, d5, d6 = ds

                # ---- va/vb/vc ----
                pA = WT("tmpA"); V.tensor_tensor(pA[:], d1[:], d4[:], alu.mult)
                pB = WT("tmpB"); V.tensor_tensor(pB[:], d3[:], d2[:], alu.mult)
                vc = WT("vc"); V.tensor_tensor(vc[:], pA[:], pB[:], alu.subtract)
                pC = WT("tmpA"); V.tensor_tensor(pC[:], d5[:], d2[:], alu.mult)
                pD = WT("tmpB"); V.tensor_tensor(pD[:], d1[:], d6[:], alu.mult)
                vb = WT("vb"); V.tensor_tensor(vb[:], pC[:], pD[:], alu.subtract)
                pE = WT("gA"); V.tensor_tensor(pE[:], d3[:], d6[:], alu.mult)
                pF = WT("gB"); V.tensor_tensor(pF[:], d5[:], d4[:], alu.mult)
                va = WT("va"); V.tensor_tensor(va[:], pE[:], pF[:], alu.subtract)

                # ---- interior v, w with NR reciprocal of fp denom ----
                dn = WT("tmpA"); V.tensor_tensor(dn[:], va[:], vb[:], alu.add)
                V.tensor_tensor(dn[:], dn[:], vc[:], alu.add)
                u = WT("tmpB"); V.tensor_tensor(u[:], dn[:], CB["RDEN"][:], alu.mult)
                V.tensor_scalar(u[:], u[:], -1.0, 2.0, alu.mult, alu.add)
                r1 = WT("tmpA"); V.tensor_tensor(r1[:], CB["RDEN"][:], u[:], alu.mult)
                v = WT("v"); V.tensor_tensor(v[:], vb[:], r1[:], alu.mult)
                w = WT("w"); V.tensor_tensor(w[:], vc[:], r1[:], alu.mult)

                # ---- edge bc ----
                tnum = WT("tnum"); V.tensor_tensor(tnum[:], d4[:], d3[:], alu.subtract)
                tden = WT("tden"); V.tensor_tensor(tden[:], d5[:], d6[:], alu.subtract)
                tt_ = WT("tt"); V.tensor_tensor(tt_[:], tnum[:], CB["RBC"][:], alu.mult)
                mm = WT("mm", bufs=2)
                V.scalar_tensor_tensor(mm[:], va[:], -1.0, tnum[:], alu.mult, alu.min)
                V.tensor_tensor(mm[:], mm[:], tden[:], alu.min)
                m = WT("m", dtype=dt.uint8); V.tensor_scalar(m[:], mm[:], 0.0, None, alu.is_ge)
                omt = WT("omt")
                V.tensor_scalar(omt[:], tt_[:], -1.0, 1.0, alu.mult, alu.add)
                V.copy_predicated(v[:], m[:], omt[:])
                V.copy_predicated(w[:], m[:], tt_[:])

                # ---- edge ac ----
                mm2 = WT("mm", bufs=2)
                V.scalar_tensor_tensor(mm2[:], vb[:], -1.0, d2[:], alu.mult, alu.min)
                V.scalar_tensor_tensor(mm2[:], d6[:], -1.0, mm2[:], alu.mult, alu.min)
                m2 = WT("m", dtype=dt.uint8); V.tensor_scalar(m2[:], mm2[:], 0.0, None, alu.is_ge)
                we = WT("wv", bufs=2); V.tensor_tensor(we[:], d2[:], CB["RAC"][:], alu.mult)
                V.copy_predicated(v[:], m2[:], zero[:])
                V.copy_predicated(w[:], m2[:], we[:])

                # ---- edge ab ----
                mm3 = WT("mm", bufs=2)
                V.scalar_tensor_tensor(mm3[:], vc[:], -1.0, d1[:], alu.mult, alu.min)
                V.scalar_tensor_tensor(mm3[:], d3[:], -1.0, mm3[:], alu.mult, alu.min)
                m3 = WT("m", dtype=dt.uint8); V.tensor_scalar(m3[:], mm3[:], 0.0, None, alu.is_ge)
                ve = WT("wv", bufs=2); V.tensor_tensor(ve[:], d1[:], CB["RAB"][:], alu.mult)
                V.copy_predicated(v[:], m3[:], ve[:])
                V.copy_predicated(w[:], m3[:], zero[:])

                # ---- vertex c ----
                mm4 = WT("mm", bufs=2)
                V.tensor_tensor(mm4[:], d6[:], d5[:], alu.subtract)
                V.tensor_tensor(mm4[:], mm4[:], d6[:], alu.min)
                m4 = WT("m", dtype=dt.uint8); V.tensor_scalar(m4[:], mm4[:], 0.0, None, alu.is_ge)
                V.copy_predicated(v[:], m4[:], zero[:])
                V.copy_predicated(w[:], m4[:], onet[:])

                # ---- vertex b ----
                mm5 = WT("mm", bufs=2)
                V.tensor_tensor(mm5[:], d3[:], d4[:], alu.subtract)
                V.tensor_tensor(mm5[:], mm5[:], d3[:], alu.min)
                m5 = WT("m", dtype=dt.uint8); V.tensor_scalar(m5[:], mm5[:], 0.0, None, alu.is_ge)
                V.copy_predicated(v[:], m5[:], onet[:])
                V.copy_predicated(w[:], m5[:], zero[:])

                # ---- vertex a ----
                mm6 = WT("mm", bufs=2)
                V.tensor_tensor(mm6[:], d1[:], d2[:], alu.max)
                m6m = WT("m", dtype=dt.uint8); V.tensor_scalar(m6m[:], mm6[:], 0.0, None, alu.is_le)
                V.copy_predicated(v[:], m6m[:], zero[:])
                V.copy_predicated(w[:], m6m[:], zero[:])

                # ---- closest point per coord + squared residual ----
                sq_tags = ["gA", "gB", "tmpB"]
                cks = []
                sqs = []
                for k in range(3):
                    ck = WT(f"ck{k}")
                    V.tensor_tensor(ck[:], v[:], ABb[k][:], alu.mult)
                    V.tensor_tensor(ck[:], ck[:], Ab[k][:], alu.add)
                    t2 = WT("t2")
                    V.tensor_tensor(t2[:], w[:], ACb[k][:], alu.mult)
                    V.tensor_tensor(ck[:], ck[:], t2[:], alu.add)
                    sq = WT(sq_tags[k])
                    # sq = Square(-ck + p_k) = (p_k - ck)^2
                    S.activation(sq[:], ck[:], AF.Square,
                                 bias=pcol[:, 3 * t + k:3 * t + k + 1],
                                 scale=-1.0)
                    cks.append(ck)
                    sqs.append(sq)

                nd = WT("tnum")
                V.tensor_tensor(nd[:], sqs[0][:], sqs[1][:], alu.add)
                # nd = (-sq2) - (sq0+sq1) = -dist2
                V.scalar_tensor_tensor(nd[:], sqs[2][:], -1.0, nd[:],
                                       alu.mult, alu.subtract)

                # ---- argmin via top-8 max of -dist2 ----
                max8 = spool.tile([128, 8], f32, tag="max8", name="max8")
                V.max(max8[:], nd[:])
                idx8 = spool.tile([128, 8], dt.uint32, tag="idx8", name="idx8")
                V.max_index(idx8[:], max8[:], nd[:])
                fx = spool.tile([128, 1], f32, tag="fx", name="fx")
                V.tensor_copy(fx[:], idx8[:, 0:1])
                oh = WT("tt")
                V.tensor_scalar(oh[:], CB["IOTA"][:], fx[:], None, alu.is_equal)

                scr = WT("omt")
                for k in range(3):
                    msk = WT("t2")
                    V.tensor_tensor(msk[:], oh[:], cks[k][:], alu.mult)
                    S.activation(scr[:], msk[:], AF.Copy,
                                 accum_out=ocp[:, 3 * t + k:3 * t + k + 1])

                S.mul(od[:, t:t + 1], max8[:, 0:1], -1.0)
                S.copy(of[:, t:t + 1], fx[:])

            G.dma_start(d_od[:], od[:])
            G.dma_start(d_ocp[:], ocp[:])
            G.dma_start(d_of[:], of[:])

    nc.compile()
    return nc


def _host_prep(triangles, points):
    """Per-triangle constants + per-core point shards (numpy fp32)."""
    f32 = np.float32
    tri = np.ascontiguousarray(triangles[0], dtype=f32)   # [F,3,3]
    pts = np.ascontiguousarray(points[0], dtype=f32)      # [P,3]

    A = tri[:, 0, :]; B = tri[:, 1, :]; C = tri[:, 2, :]
    AB = B - A
    AC = C - A

    def dot3(x, y):
        t = x * y
        return (t[:, 0] + t[:, 1]) + t[:, 2]

    ABdA = dot3(AB, A); ACdA = dot3(AC, A)
    ABdB = dot3(AB, B); ACdB = dot3(AC, B)
    ABdC = dot3(AB, C); ACdC = dot3(AC, C)

    AB64 = AB.astype(np.float64); AC64 = AC.astype(np.float64)
    BC64 = (C - B).astype(np.float64)
    cr = np.cross(AB64, AC64)
    R_AB = (1.0 / (AB64 ** 2).sum(1)).astype(f32)
    R_AC = (1.0 / (AC64 ** 2).sum(1)).astype(f32)
    R_BC = (1.0 / (BC64 ** 2).sum(1)).astype(f32)
    R_DEN = (1.0 / (cr ** 2).sum(1)).astype(f32)

    crows = np.zeros((16, F), f32)
    for i, row in enumerate([AB[:, 0], AB[:, 1], AB[:, 2],
                             AC[:, 0], AC[:, 1], AC[:, 2],
                             A[:, 0], A[:, 1], A[:, 2],
                             R_AB, R_AC, R_BC, R_DEN,
                             np.arange(F, dtype=f32)]):
        crows[i] = row

    m6 = np.zeros((24, F), f32)
    mats = [(AB, ABdA), (AC, ACdA), (AB, ABdB), (AC, ACdB), (AB, ABdC), (AC, ACdC)]
    for j, (E, c) in enumerate(mats):
        m6[4 * j + 0] = E[:, 0]
        m6[4 * j + 1] = E[:, 1]
        m6[4 * j + 2] = E[:, 2]
        m6[4 * j + 3] = -c

    in_maps = []
    for cidx in range(N_CORES):
        pc = pts[cidx * P_LOC:(cidx + 1) * P_LOC]          # [2048,3]
        ptsT = np.empty((4, P_LOC), f32)
        ptsT[0] = pc[:, 0]; ptsT[1] = pc[:, 1]; ptsT[2] = pc[:, 2]; ptsT[3] = 1.0
        pcol = np.empty((128, 3 * PTILES), f32)
        for t in range(PTILES):
            blk = pc[t * 128:(t + 1) * 128]                # [128,3]
            pcol[:, 3 * t:3 * t + 3] = blk
        in_maps.append({
            "crows": crows, "m6": m6, "ptsT": ptsT, "pcol": pcol,
        })
    return in_maps


def kernel(triangles, points):
    from concourse.bass_utils import run_bass_kernel_spmd

    if "nc" not in _PROGRAM_CACHE:
        _PROGRAM_CACHE["nc"] = _build_program()
    nc = _PROGRAM_CACHE["nc"]

    in_maps = _host_prep(triangles, points)
    res = run_bass_kernel_spmd(nc, in_maps, list(range(N_CORES)))

    distances = np.empty((1, P_TOTAL), np.float32)
    closest_points = np.empty((1, P_TOTAL, 3), np.float32)
    closest_faces = np.empty((1, P_TOTAL), np.int32)
    for cidx in range(N_CORES):
        r = res.results[cidx]
        od, ocp, of = r["od"], r["ocp"], r["of"]
        base = cidx * P_LOC
        for t in range(PTILES):
            sl = slice(base + t * 128, base + (t + 1) * 128)
            distances[0, sl] = od[:, t]
            closest_points[0, sl, :] = ocp[:, 3 * t:3 * t + 3]
            closest_faces[0, sl] = of[:, t].astype(np.int32)
    return distances, closest_points, closest_faces
